# revision 1
# baseline (speedup 1.0000x reference)
"""Trainium2 Bass kernel for nn_LongTermEncoder (gnn_message_passing).

Sharding: data-parallel over batch B=8 across the 8 NeuronCores (adjacency and
all parameters replicated), per the sharding hint.  The whole 3-layer network
(inception convs, gating, mixprop channel projections, the four dense
[1000x1000] adjacency applies per layer, and layernorm) runs on-device in a
single bass_jit(target_bir_lowering=True) SPMD program; the host computes only
the graph constructor (top-k + softmax), the 1x1 start conv, weight folding,
and the final adaptive average pool.

mixprop is refactored exactly (channel mixing commutes with node mixing):
  dir1: out = Q0 x + A(Q1 x + A(Q2 x)),  A = (adp+I)/2   [adp rows sum to 1]
  dir2: out = R0 x + B(R1 x + B(R2 x)),  B = D^-1 (adp^T + I)
folded to the 5 device projections p0=(Q0+R0)x, m1=(Q1+.5Q2)x, c2=Q2x,
q1=R1x, q2=R2x.
"""
import threading
import numpy as np

L, GDEP, PA, ALPHA, KTOP, TSHORT, EPS = 3, 2, 0.05, 3.0, 20, 12, 1e-5
KSET = (2, 4, 6, 8)
N, B, RC, CC = 1000, 8, 8, 32
NP_ = 1024                     # padded node count
f32 = np.float32


# ---------------- host math (validated vs reference) ----------------
def _graph_prep(d):
    emb1, emb2 = d["emb1"], d["emb2"]
    v1 = np.tanh(ALPHA * (emb1 @ d["lin1_w"].T + d["lin1_b"])).astype(f32)
    v2 = np.tanh(ALPHA * (emb2 @ d["lin2_w"].T + d["lin2_b"])).astype(f32)
    a = v1 @ v2.T - v2 @ v1.T
    adj = np.maximum(np.tanh(ALPHA * a), 0.0).astype(f32)
    score = adj + f32(0.01) * d["topk_noise"]
    t1 = np.argpartition(-score, KTOP - 1, axis=1)[:, :KTOP]
    mask = np.zeros((N, N), f32)
    np.put_along_axis(mask, t1, 1.0, axis=1)
    adp = adj * mask
    mv = (1.0 - d["cooldowns"]).astype(f32)
    z = adp * (mv[:, None] * mv[None, :])
    z = z - z.max(axis=1, keepdims=True)
    e = np.exp(z)
    return (e / e.sum(axis=1, keepdims=True)).astype(f32)


def _fold_proj(d, l):
    W = d["g1_w"][l].astype(f32)
    W0, W1, W2 = W[:, :32], W[:, 32:64], W[:, 64:]
    V = d["g2_w"][l].astype(f32)
    V0, V1, V2 = V[:, :32], V[:, 32:64], V[:, 64:]
    g = 1.0 - PA
    Q0 = W0 + PA * (W1 + W2)
    Q1 = g * (W1 + PA * W2)
    Q2 = g * g * W2
    R0 = V0 + PA * (V1 + V2)
    R1 = g * (V1 + PA * V2)
    R2 = g * g * V2
    # [32 in, 40 out] column blocks [p0 | m1 | c2 | q1 | q2]
    return np.concatenate([(Q0 + R0).T, (Q1 + 0.5 * Q2).T, Q2.T, R1.T, R2.T],
                          axis=1).astype(f32)


def _fold_conv(d, l):
    # lhsT [64 rows=(d,c), 64 cols = filt32|gate32]
    w = np.zeros((64, 64), f32)
    for half, pre in ((0, "f"), (32, "g")):
        for bi, k in enumerate(KSET):
            wk = d[pre + "w%d" % k][l]          # [8, 8, 1, k]
            for dd in range(8 - k, 8):
                j = dd - (8 - k)
                # rows p = dd*8 + c ; cols = half + bi*8 + o
                w[dd * 8:(dd + 1) * 8, half + bi * 8:half + bi * 8 + 8] = \
                    wk[:, :, 0, j].T
    return w


# ---------------- device program (path-stable source) ----------------
_DEV_SRC = r'''
import numpy as _np
import jax
import ml_dtypes
import concourse.bass as bass
import concourse.mybir as mybir
from concourse.tile import TileContext
from concourse.bass2jax import bass_jit
from jax.sharding import Mesh, PartitionSpec as P
from jax.experimental.shard_map import shard_map

bf = mybir.dt.bfloat16
fp = mybir.dt.float32
MUL = mybir.AluOpType.mult
ADD = mybir.AluOpType.add
AF = mybir.ActivationFunctionType
NP_ = 1024
TS = (168, 161, 154)
TPS = (161, 154, 147)
CH = 512                      # psum chunk (fp32 bank)
NEL = (8 * 1000 * 161, 8 * 1000 * 154, 8 * 1000 * 147)
DBG = False


def ltenc(nc: bass.Bass, xc, ident, adp, dmat, ones2, orow, wcv, wpj):
    # xc   [8, 1024*168] bf16 channel-major start-conv output (one batch elem)
    # ident [128, 128] bf16 identity ; adp [1024, 1024] bf16
    # dmat [128, 8] f32 dinv by (partition, vtile)
    # ones2 [128, 2] f32 (col0 ones, col1 ones masked to first 104 rows)
    # orow [1, 128] f32 ones ; wcv [192, 64] bf16 ; wpj [96, 40] bf16
    out = nc.dram_tensor("xout", (1000, 8 * 147), bf, kind="ExternalOutput")
    if DBG:
        dpa = nc.dram_tensor("dpa", (NP_, 40 * 161), bf, kind="ExternalOutput")
        du1 = nc.dram_tensor("du1", (NP_, 8 * 161), bf, kind="ExternalOutput")
        duf = nc.dram_tensor("duf", (NP_, 8 * 161), bf, kind="ExternalOutput")
        dst = nc.dram_tensor("dst", (128, 48), fp, kind="ExternalOutput")
        dab = nc.dram_tensor("dab", (128, 2), fp, kind="ExternalOutput")
        dx1 = nc.dram_tensor("dx1", (8, NP_ * 161), bf, kind="ExternalOutput")
        dxa = nc.dram_tensor("dxa", (NP_, 8 * 161), bf, kind="ExternalOutput")
    with TileContext(nc) as tc:
        with tc.tile_pool(name="const", bufs=1) as cp, \
             tc.tile_pool(name="dram", bufs=1, space="DRAM") as dr, \
             tc.tile_pool(name="adj", bufs=1) as aj, \
             tc.tile_pool(name="wk", bufs=2) as wk, \
             tc.tile_pool(name="st", bufs=1) as stp:
            dv = cp.tile([128, 8], fp, tag="dv", name="dv")
            nc.sync.dma_start(out=dv[:, :], in_=dmat[:, :])
            o2 = cp.tile([128, 2], fp, tag="o2", name="o2")
            nc.sync.dma_start(out=o2[:, :], in_=ones2[:, :])
            orw = cp.tile([1, 128], fp, tag="orw", name="orw")
            nc.sync.dma_start(out=orw[:, :], in_=orow[:, :])
            wc = []
            wp = []
            for l in range(3):
                t = cp.tile([64, 64], bf, tag="wc%d" % l, name="wc%d" % l)
                nc.sync.dma_start(out=t[:, :], in_=wcv[l * 64:(l + 1) * 64, :])
                wc.append(t)
                t = cp.tile([32, 40], bf, tag="wp%d" % l, name="wp%d" % l)
                nc.sync.dma_start(out=t[:, :], in_=wpj[l * 32:(l + 1) * 32, :])
                wp.append(t)

            adpT = dr.tile([NP_, NP_], bf, tag="adpT", name="adpT")
            idt = cp.tile([128, 128], bf, tag="idt", name="idt")
            nc.sync.dma_start(out=idt[:, :], in_=ident[:, :])
            with tc.tile_pool(name="trp", bufs=2) as trp, \
                 tc.tile_pool(name="pst", bufs=2, space="PSUM") as pst:
                for ki in range(8):
                    sa = trp.tile([128, NP_], bf, tag="sa", name="sa")
                    nc.sync.dma_start(out=sa[:, :],
                                      in_=adp[ki * 128:(ki + 1) * 128, :])
                    for vj in range(8):
                        pt = pst.tile([128, 128], bf, tag="pt", name="pt")
                        nc.tensor.transpose(pt[:, :],
                                            sa[:, vj * 128:(vj + 1) * 128],
                                            idt[:, :])
                        so = trp.tile([128, 128], bf, tag="so", name="so")
                        nc.vector.tensor_copy(so[:, :], pt[:, :])
                        nc.sync.dma_start(
                            out=adpT[vj * 128:(vj + 1) * 128,
                                     ki * 128:(ki + 1) * 128],
                            in_=so[:, :])

            xcd = [None,
                   dr.tile([8, NP_ * 161], bf, tag="xc1", name="xc1"),
                   dr.tile([8, NP_ * 154], bf, tag="xc2", name="xc2")]
            pA = dr.tile([NP_, 40 * 161], bf, tag="pA", name="pA")
            uA1 = dr.tile([NP_, 8 * 161], bf, tag="uA1", name="uA1")
            uAf = dr.tile([NP_, 8 * 161], bf, tag="uAf", name="uAf")

            for l in range(3):
                T, Tp, F = TS[l], TPS[l], 8 * TPS[l]
                xin = xc if l == 0 else xcd[l]
                xin3 = xin.rearrange("c (n t) -> c n t", t=T)
                pA3 = pA[:, :40 * Tp].rearrange("n (o t) -> o n t", o=40)

                # ---- S1: inception conv + gate + channel projections ----
                with tc.tile_pool(name="s1", bufs=2) as s1p, \
                     tc.tile_pool(name="ps1", bufs=2, space="PSUM") as ps1:
                    for v in range(16):           # 64-node subtiles
                        n0 = v * 64
                        FS = 64 * Tp
                        xs = s1p.tile([64, FS], bf, tag="xs", name="xs")
                        for dd in range(8):
                            nc.sync.dma_start(
                                out=xs[dd * 8:(dd + 1) * 8, :],
                                in_=xin3[:, n0:n0 + 64, dd:dd + Tp])
                        pc = s1p.tile([40, FS], bf, tag="pc", name="pc")
                        nch = (FS + CH - 1) // CH
                        for c in range(nch):
                            c0 = c * CH
                            w = min(CH, FS - c0)
                            ps = ps1.tile([64, CH], fp, tag="cps", name="cps")
                            nc.tensor.matmul(ps[:, :w], wc[l][:, :],
                                             xs[:, c0:c0 + w],
                                             start=True, stop=True)
                            tf = s1p.tile([32, CH], bf, tag="tf", name="tf")
                            nc.scalar.activation(tf[:, :w], ps[:32, :w], AF.Tanh)
                            tg = s1p.tile([32, CH], bf, tag="tg", name="tg")
                            nc.scalar.activation(tg[:, :w], ps[32:64, :w], AF.Sigmoid)
                            x1 = s1p.tile([32, CH], bf, tag="x1", name="x1")
                            nc.vector.tensor_mul(x1[:, :w], tf[:, :w], tg[:, :w])
                            pp = ps1.tile([40, CH], fp, tag="pps", name="pps")
                            nc.tensor.matmul(pp[:, :w], wp[l][:, :],
                                             x1[:, :w], start=True, stop=True)
                            nc.vector.tensor_copy(pc[:, c0:c0 + w], pp[:, :w])
                        nc.sync.dma_start(out=pA3[:, n0:n0 + 64, :],
                                          in_=pc[:, :])

                if DBG and l == 0:
                    nc.sync.dma_start(out=dpa[:, :], in_=pA[:, :])
                # chunk plan for F = 8*Tp
                chs = []
                c0 = 0
                while c0 < F:
                    chs.append((c0, min(CH, F - c0)))
                    c0 += CH
                blk = lambda b: slice(b * 8 * Tp, (b + 1) * 8 * Tp)  # noqa: E731

                sA = stp.tile([128, 42], fp, tag="sA", name="sA")   # (v<7)*3ch x {sum,sq}
                sB = stp.tile([128, 6], fp, tag="sB", name="sB")    # v=7
                nc.vector.memset(sA[:, :], 0)
                nc.vector.memset(sB[:, :], 0)

                with tc.tile_pool(name="ps2", bufs=3, space="PSUM") as ps2:
                    # ---- S2: dir1 pass1: s1 = 0.5 z2 + m1 ----
                    ct = [aj.tile([128, F], bf, tag="c_%d" % k, name="c_%d" % k) for k in range(8)]
                    for k in range(8):
                        nc.sync.dma_start(out=ct[k][:, :],
                                          in_=pA[k * 128:(k + 1) * 128, blk(2)])
                    s1t = [aj.tile([128, F], bf, tag="s_%d" % k, name="s_%d" % k) for k in range(8)]
                    for v in range(8):
                        strip = wk.tile([128, 1024], bf, tag="strip", name="strip")
                        nc.sync.dma_start(
                            out=strip[:, :],
                            in_=adpT.rearrange("(k p) w -> p k w", p=128)[:, :, v * 128:(v + 1) * 128])
                        pv = wk.tile([128, F], bf, tag="pv", name="pv")
                        nc.sync.dma_start(out=pv[:, :],
                                          in_=pA[v * 128:(v + 1) * 128, blk(1)])
                        for (c0, w) in chs:
                            zp = ps2.tile([128, CH], fp, tag="zp", name="zp")
                            for k in range(8):
                                nc.tensor.matmul(zp[:, :w],
                                                 strip[:, k * 128:(k + 1) * 128],
                                                 ct[k][:, c0:c0 + w],
                                                 start=(k == 0), stop=(k == 7))
                            nc.vector.scalar_tensor_tensor(
                                s1t[v][:, c0:c0 + w], zp[:, :w], 0.5,
                                pv[:, c0:c0 + w], op0=MUL, op1=ADD)

                    # ---- S3: dir1 pass2: u1 = 0.5 z1 + p0 + 0.5 s1 ----
                    for v in range(8):
                        strip = wk.tile([128, 1024], bf, tag="strip", name="strip")
                        nc.sync.dma_start(
                            out=strip[:, :],
                            in_=adpT.rearrange("(k p) w -> p k w", p=128)[:, :, v * 128:(v + 1) * 128])
                        pv = wk.tile([128, F], bf, tag="pv", name="pv")
                        nc.sync.dma_start(out=pv[:, :],
                                          in_=pA[v * 128:(v + 1) * 128, blk(0)])
                        u1v = wk.tile([128, F], bf, tag="u1v", name="u1v")
                        for (c0, w) in chs:
                            zp = ps2.tile([128, CH], fp, tag="zp", name="zp")
                            for k in range(8):
                                nc.tensor.matmul(zp[:, :w],
                                                 strip[:, k * 128:(k + 1) * 128],
                                                 s1t[k][:, c0:c0 + w],
                                                 start=(k == 0), stop=(k == 7))
                            w1 = wk.tile([128, CH], fp, tag="w1", name="w1")
                            nc.vector.scalar_tensor_tensor(
                                w1[:, :w], zp[:, :w], 0.5, pv[:, c0:c0 + w],
                                op0=MUL, op1=ADD)
                            nc.vector.scalar_tensor_tensor(
                                u1v[:, c0:c0 + w], s1t[v][:, c0:c0 + w], 0.5,
                                w1[:, :w], op0=MUL, op1=ADD)
                        nc.sync.dma_start(out=uA1[v * 128:(v + 1) * 128, :F],
                                          in_=u1v[:, :])

                    if DBG and l == 0:
                        nc.sync.dma_start(out=du1[:, :F], in_=uA1[:, :F])
                    # ---- S4: dir2 pass1: s2 = dinv z2' + (q1 + dinv q2) ----
                    for k in range(8):
                        nc.sync.dma_start(out=ct[k][:, :],
                                          in_=pA[k * 128:(k + 1) * 128, blk(4)])
                    for v in range(8):
                        strip = wk.tile([128, 1024], bf, tag="strip", name="strip")
                        nc.sync.dma_start(
                            out=strip[:, :],
                            in_=adp.rearrange("(k p) w -> p k w", p=128)[:, :, v * 128:(v + 1) * 128])
                        pv = wk.tile([128, F], bf, tag="pv", name="pv")
                        nc.sync.dma_start(out=pv[:, :],
                                          in_=pA[v * 128:(v + 1) * 128, blk(3)])
                        q12 = wk.tile([128, F], bf, tag="q12", name="q12")
                        nc.vector.scalar_tensor_tensor(
                            q12[:, :], ct[v][:, :], dv[:, v:v + 1], pv[:, :],
                            op0=MUL, op1=ADD)
                        for (c0, w) in chs:
                            zp = ps2.tile([128, CH], fp, tag="zp", name="zp")
                            for k in range(8):
                                nc.tensor.matmul(zp[:, :w],
                                                 strip[:, k * 128:(k + 1) * 128],
                                                 ct[k][:, c0:c0 + w],
                                                 start=(k == 0), stop=(k == 7))
                            nc.vector.scalar_tensor_tensor(
                                s1t[v][:, c0:c0 + w], zp[:, :w], dv[:, v:v + 1],
                                q12[:, c0:c0 + w], op0=MUL, op1=ADD)

                    # ---- S5: dir2 pass2: u = u1 + dinv (z1' + s2) + res ----
                    for v in range(8):
                        strip = wk.tile([128, 1024], bf, tag="strip", name="strip")
                        nc.sync.dma_start(
                            out=strip[:, :],
                            in_=adp.rearrange("(k p) w -> p k w", p=128)[:, :, v * 128:(v + 1) * 128])
                        u1v = wk.tile([128, F], bf, tag="u1v", name="u1v")
                        nc.sync.dma_start(out=u1v[:, :],
                                          in_=uA1[v * 128:(v + 1) * 128, :F])
                        rsv = wk.tile([128, F], bf, tag="rsv", name="rsv")
                        nc.sync.dma_start(
                            out=rsv[:, :],
                            in_=xin.rearrange("c (n t) -> n c t", t=T)[v * 128:(v + 1) * 128, :, T - Tp:])
                        uv = wk.tile([128, F], bf, tag="uv", name="uv")
                        for ci, (c0, w) in enumerate(chs):
                            zp = ps2.tile([128, CH], fp, tag="zp", name="zp")
                            for k in range(8):
                                nc.tensor.matmul(zp[:, :w],
                                                 strip[:, k * 128:(k + 1) * 128],
                                                 s1t[k][:, c0:c0 + w],
                                                 start=(k == 0), stop=(k == 7))
                            w1 = wk.tile([128, CH], fp, tag="w1", name="w1")
                            nc.vector.tensor_add(w1[:, :w], zp[:, :w],
                                                 s1t[v][:, c0:c0 + w])
                            w2 = wk.tile([128, CH], fp, tag="w2", name="w2")
                            nc.vector.scalar_tensor_tensor(
                                w2[:, :w], w1[:, :w], dv[:, v:v + 1],
                                u1v[:, c0:c0 + w], op0=MUL, op1=ADD)
                            uvf = wk.tile([128, CH], fp, tag="uvf", name="uvf")
                            if v < 7:
                                so = sA[:, (v * 3 + ci):(v * 3 + ci) + 1]
                                qo = sA[:, (21 + v * 3 + ci):(21 + v * 3 + ci) + 1]
                            else:
                                so = sB[:, ci:ci + 1]
                                qo = sB[:, 3 + ci:3 + ci + 1]
                            nc.vector.scalar_tensor_tensor(
                                uvf[:, :w], w2[:, :w], 1.0, rsv[:, c0:c0 + w],
                                op0=MUL, op1=ADD, accum_out=so)
                            scr = wk.tile([128, CH], fp, tag="scr", name="scr")
                            nc.scalar.activation(scr[:, :w], uvf[:, :w],
                                                 AF.Square, accum_out=qo)
                            nc.vector.tensor_copy(uv[:, c0:c0 + w], uvf[:, :w])
                        nc.sync.dma_start(out=uAf[v * 128:(v + 1) * 128, :F],
                                          in_=uv[:, :])

                if DBG and l == 0:
                    nc.sync.dma_start(out=duf[:, :F], in_=uAf[:, :F])
                    nc.sync.dma_start(out=dst[:, 0:42], in_=sA[:, :])
                    nc.sync.dma_start(out=dst[:, 42:48], in_=sB[:, :])
                # ---- S6: global layernorm stats ----
                with tc.tile_pool(name="ps3", bufs=1, space="PSUM") as ps3:
                    rA = ps3.tile([1, 42], fp, tag="rA", name="rA")
                    nc.tensor.matmul(rA[:, :], o2[:, 0:1], sA[:, :],
                                     start=True, stop=True)
                    rB = ps3.tile([1, 6], fp, tag="rB", name="rB")
                    nc.tensor.matmul(rB[:, :], o2[:, 1:2], sB[:, :],
                                     start=True, stop=True)
                    sc = stp.tile([1, 64], fp, tag="sc", name="sc")
                    nc.vector.tensor_copy(sc[:, 0:42], rA[:, :])
                    nc.vector.tensor_copy(sc[:, 42:48], rB[:, :])
                    # sums: cols 0:21 and 42:45 ; sqs: 21:42 and 45:48
                    nc.vector.reduce_sum(sc[:, 48:49], sc[:, 0:21],
                                         axis=mybir.AxisListType.X)
                    nc.vector.reduce_sum(sc[:, 49:50], sc[:, 42:45],
                                         axis=mybir.AxisListType.X)
                    nc.vector.reduce_sum(sc[:, 50:51], sc[:, 21:42],
                                         axis=mybir.AxisListType.X)
                    nc.vector.reduce_sum(sc[:, 51:52], sc[:, 45:48],
                                         axis=mybir.AxisListType.X)
                    nc.vector.tensor_add(sc[:, 52:53], sc[:, 48:49], sc[:, 49:50])
                    nc.vector.tensor_add(sc[:, 53:54], sc[:, 50:51], sc[:, 51:52])
                    inel = 1.0 / NEL[l]
                    nc.scalar.mul(sc[:, 54:55], sc[:, 52:53], inel)   # mean
                    nc.scalar.activation(sc[:, 55:56], sc[:, 53:54], AF.Copy,
                                         bias=1e-5, scale=inel)       # E[x^2]+eps
                    nc.scalar.activation(sc[:, 56:57], sc[:, 54:55], AF.Square)
                    nc.vector.scalar_tensor_tensor(
                        sc[:, 57:58], sc[:, 56:57], -1.0, sc[:, 55:56],
                        op0=MUL, op1=ADD)                             # var
                    nc.scalar.activation(sc[:, 58:59], sc[:, 57:58],
                                         AF.Sqrt)                     # sqrt(var+eps)
                    nc.vector.reciprocal(sc[:, 59:60], sc[:, 58:59])  # inv
                    nc.vector.tensor_mul(sc[:, 60:61], sc[:, 54:55], sc[:, 59:60])
                    nc.scalar.mul(sc[:, 61:62], sc[:, 60:61], -1.0)   # -mean*inv
                    ab = stp.tile([1, 2], fp, tag="ab", name="ab")
                    nc.vector.tensor_copy(ab[:, 0:1], sc[:, 59:60])
                    nc.vector.tensor_copy(ab[:, 1:2], sc[:, 61:62])
                    pb = ps3.tile([128, 2], fp, tag="pb", name="pb")
                    nc.tensor.matmul(pb[:, :], orw[:, :], ab[:, :],
                                     start=True, stop=True)
                    abb = stp.tile([128, 2], fp, tag="abb", name="abb")
                    nc.vector.tensor_copy(abb[:, :], pb[:, :])
                    if DBG and l == 0:
                        nc.sync.dma_start(out=dab[:, :], in_=abb[:, :])

                # ---- S7: normalize + write next-layer input ----
                if l < 2:
                    xnx3 = xcd[l + 1].rearrange("c (n t) -> n c t", t=Tp)
                for v in range(8):
                    uv = wk.tile([128, F], bf, tag="uv", name="uv")
                    nc.sync.dma_start(out=uv[:, :],
                                      in_=uAf[v * 128:(v + 1) * 128, :F])
                    xv = wk.tile([128, F], bf, tag="xv", name="xv")
                    nc.scalar.activation(xv[:, :], uv[:, :], AF.Identity,
                                         bias=abb[:, 1:2], scale=abb[:, 0:1])
                    if DBG and l == 0:
                        nc.sync.dma_start(out=dxa[v * 128:(v + 1) * 128, :F],
                                          in_=xv[:, :])
                    if l < 2:
                        nc.sync.dma_start(
                            out=xnx3[v * 128:(v + 1) * 128, :, :],
                            in_=xv[:, :])
                    elif v < 7:
                        nc.sync.dma_start(out=out[v * 128:(v + 1) * 128, :],
                                          in_=xv[:, :])
                    else:
                        nc.sync.dma_start(out=out[896:1000, :],
                                          in_=xv[:104, :])
                if DBG and l == 0:
                    nc.sync.dma_start(out=dx1[:, :], in_=xcd[1][:, :])
    if DBG:
        return out, dpa, du1, duf, dst, dab, dx1, dxa
    return out


IN_SPECS = None


def _specs():
    global IN_SPECS
    if IN_SPECS is None:
        pr = P("x", None)
        pn = P(None, None)
        IN_SPECS = (pr, pn, pn, pn, pn, pn, pn, pn)
    return IN_SPECS


def make_runner(mesh):
    kern = bass_jit(ltenc, target_bir_lowering=True,
                    disable_frame_to_traceback=True)

    def call(xc, ident, adp, dmat, ones2, orow, wcv, wpj):
        return kern(xc, ident, adp, dmat, ones2, orow, wcv, wpj)

    sm = shard_map(call, mesh=mesh, in_specs=_specs(),
                   out_specs=P("x", None), check_rep=False)
    return jax.jit(sm)


def _patch_effect():
    import concourse.bass2jax as _b2j
    _b2j.BassEffect.__eq__ = lambda self, other: type(self) is type(other)
    _b2j.BassEffect.__hash__ = lambda self: hash(type(self))
    _b2j.install_neuronx_cc_hook()


def get_callable():
    import os
    import jax.export
    from jax.sharding import NamedSharding
    _patch_effect()
    mesh = Mesh(_np.array(jax.devices()[:8]), ("x",))
    cdir = "/root/.cache/ltenc"
    path = os.path.join(cdir, "ltenc_%s.expbin" % SRC_HASH)
    blob = None
    if os.path.exists(path):
        try:
            blob = open(path, "rb").read()
        except OSError:
            blob = None
    if blob is None:
        runner = make_runner(mesh)
        import ml_dtypes
        shp = [jax.ShapeDtypeStruct((64, 1024 * 168), ml_dtypes.bfloat16),
               jax.ShapeDtypeStruct((128, 128), ml_dtypes.bfloat16),
               jax.ShapeDtypeStruct((1024, 1024), ml_dtypes.bfloat16),
               jax.ShapeDtypeStruct((128, 8), _np.float32),
               jax.ShapeDtypeStruct((128, 2), _np.float32),
               jax.ShapeDtypeStruct((1, 128), _np.float32),
               jax.ShapeDtypeStruct((192, 64), ml_dtypes.bfloat16),
               jax.ShapeDtypeStruct((96, 40), ml_dtypes.bfloat16)]
        dc = [jax.export.DisabledSafetyCheck.custom_call("bass_exec"),
              jax.export.DisabledSafetyCheck.custom_call(
                  "AwsNeuronCustomNativeKernel")]
        exp = jax.export.export(runner, disabled_checks=dc)(*shp)
        blob = exp.serialize()
        try:
            os.makedirs(cdir, exist_ok=True)
            tmp = path + ".tmp.%d" % os.getpid()
            with open(tmp, "wb") as fh:
                fh.write(blob)
            os.replace(tmp, path)
        except OSError:
            pass
    exp2 = jax.export.deserialize(blob)
    shard = [NamedSharding(mesh, sp) for sp in _specs()]
    return jax.jit(exp2.call, in_shardings=shard)


HOLDER = {}


def thread_main():
    try:
        args = HOLDER["args"]
        if "runner" not in HOLDER:
            HOLDER["runner"] = get_callable()
        r = HOLDER["runner"](*args)
        HOLDER["out"] = _np.asarray(r)
    except BaseException as e:  # noqa: BLE001
        import traceback
        traceback.print_exc()
        HOLDER["err"] = e
'''

_ENV = {"ns": None, "fail": False, "compiled": None, "thread": None,
        "args": None, "out": None, "err": None}
_ARGS_READY = threading.Event()
_SHAPES = [((64, 1024 * 168), "bfloat16"), ((128, 128), "bfloat16"),
           ((1024, 1024), "bfloat16"), ((128, 8), "float32"),
           ((128, 2), "float32"), ((1, 128), "float32"),
           ((192, 64), "bfloat16"), ((96, 40), "bfloat16")]


def _setup_and_run():
    try:
        import hashlib
        import numpy as _np
        if _ENV["ns"] is None:
            ns = {}
            exec(compile(_DEV_SRC, "<ltenc>", "exec"), ns)
            ns["SRC_HASH"] = hashlib.sha256(_DEV_SRC.encode()).hexdigest()[:16]
            _ENV["ns"] = ns
        ns = _ENV["ns"]
        if _ENV["compiled"] is None:
            fn = ns["get_callable"]()
            import jax
            import ml_dtypes
            dt = {"bfloat16": ml_dtypes.bfloat16, "float32": np.float32}
            shp = [jax.ShapeDtypeStruct(sh, dt[t]) for sh, t in _SHAPES]
            _ENV["compiled"] = fn.lower(*shp).compile()
        _ARGS_READY.wait(timeout=600.0)
        if _ENV["args"] is None:
            return
        r = _ENV["compiled"](*_ENV["args"])
        _ENV["out"] = np.asarray(r)
    except BaseException as e:  # noqa: BLE001
        import traceback
        traceback.print_exc()
        _ENV["err"] = e


def _device_begin():
    if _ENV["fail"] or _ENV["thread"] is not None:
        return
    _ENV["err"] = None
    _ENV["out"] = None
    t = threading.Thread(target=_setup_and_run, daemon=True)
    _ENV["thread"] = t
    t.start()


def _device_forward(*args):
    """Feed args to the setup thread and wait. Returns output or None."""
    import hashlib
    import os
    if _ENV["fail"]:
        return None
    try:
        h = hashlib.sha256(_DEV_SRC.encode()).hexdigest()[:16]
        warm = os.path.exists("/root/.cache/ltenc/ltenc_%s.expbin" % h)
        for attempt in range(2):
            if _ENV["thread"] is None:
                _device_begin()
            _ENV["args"] = args
            _ARGS_READY.set()
            _ENV["thread"].join(timeout=45.0 if warm else 900.0)
            alive = _ENV["thread"].is_alive()
            _ENV["thread"] = None
            _ARGS_READY.clear()
            _ENV["args"] = None
            if alive:
                break
            if _ENV["err"] is None and _ENV["out"] is not None:
                return _ENV["out"]
            if attempt == 0:
                _ENV["err"] = None
                _ENV["out"] = None
        raise RuntimeError("device path failed or timed out")
    except Exception:
        import traceback
        traceback.print_exc()
        _ENV["fail"] = True
        return None


# ---------------- full forward ----------------
def _host_fallback(d, adp):
    # exact folded host math (fast BLAS path; used only if the device fails
    # or the network uses non-default biases / norm params)
    general = (d["g1_b"].any() or d["g2_b"].any() or d["start_b"].any()
               or any(d[p + "b%d" % k].any() for p in ("f", "g") for k in KSET)
               or any(d["nb%d" % j].any() for j in (1, 2, 3))
               or not all((d["nw%d" % j] == 1.0).all() for j in (1, 2, 3)))
    dinv = (1.0 / (1.0 + adp.sum(axis=0))).astype(f32)
    x = (np.einsum("oi,bint->bont", d["start_w"], d["input"], optimize=True)
         + d["start_b"][None, :, None, None]).astype(f32)
    di = dinv[None, None, :, None]
    for l in range(L):
        T = x.shape[-1]
        Tp = T - 7
        xs = np.empty((64, B, N, Tp), f32)
        for dd in range(8):
            xs[dd * 8:(dd + 1) * 8] = x[:, :, :, dd:dd + Tp].transpose(1, 0, 2, 3)
        wcl = _fold_conv(d, l)
        conv = np.einsum("ko,kbnt->obnt", wcl, xs, optimize=True)
        if general:
            cb = np.zeros((64,), f32)
            for half, pre in ((0, "f"), (32, "g")):
                for bi, k in enumerate(KSET):
                    cb[half + bi * 8:half + bi * 8 + 8] = d[pre + "b%d" % k][l]
            conv = conv + cb[:, None, None, None]
        filt = np.tanh(conv[:32])
        gate = 1.0 / (1.0 + np.exp(-conv[32:]))
        x1 = (filt * gate).astype(f32)
        proj = np.einsum("co,cbnt->obnt", _fold_proj(d, l), x1, optimize=True)
        p0, m1, c2, q1, q2 = (proj[i * 8:(i + 1) * 8] for i in range(5))
        z2 = np.einsum("vw,obwt->obvt", adp, c2, optimize=True)
        s1 = 0.5 * z2 + m1
        z1 = np.einsum("vw,obwt->obvt", adp, s1, optimize=True)
        u1 = 0.5 * z1 + p0 + 0.5 * s1
        z2b = np.einsum("wv,obwt->obvt", adp, q2, optimize=True)
        s2 = di * z2b + (q1 + di * q2)
        z1b = np.einsum("wv,obwt->obvt", adp, s2, optimize=True)
        u = u1 + di * (z1b + s2)
        u = (u + x.transpose(1, 0, 2, 3)[:, :, :, T - Tp:]).transpose(1, 0, 2, 3)
        if general:
            u = u + (d["g1_b"][l] + d["g2_b"][l])[None, :, None, None]
        mu = u.mean(axis=(1, 2, 3), keepdims=True)
        var = u.var(axis=(1, 2, 3), keepdims=True)
        x = ((u - mu) / np.sqrt(var + EPS)).astype(f32)
        if general:
            x = (x * d["nw%d" % (l + 1)][None] + d["nb%d" % (l + 1)][None]).astype(f32)
    return x


def _pool(x):
    T = x.shape[-1]
    p = np.zeros((TSHORT, T), f32)
    for i in range(TSHORT):
        s = (i * T) // TSHORT
        e = -((-(i + 1) * T) // TSHORT)
        p[i, s:e] = 1.0 / (e - s)
    return np.einsum("st,bcnt->bcsn", p, x).astype(f32)


def kernel(**d):
    import ml_dtypes
    bf16 = ml_dtypes.bfloat16
    _device_begin()
    d = {k: np.asarray(v) for k, v in d.items()}
    adp = _graph_prep(d)

    # the device path folds biases/norm params assuming the reference setup
    simple = (all(not d[p + "b%d" % k][...].any() for p in ("f", "g") for k in KSET)
              and not d["g1_b"].any() and not d["g2_b"].any()
              and not d["start_b"].any()
              and all(not d["nb%d" % j].any() for j in (1, 2, 3))
              and all((d["nw%d" % j] == 1.0).all() for j in (1, 2, 3)))

    xf = None
    if simple:
        dinv = (1.0 / (1.0 + adp.sum(axis=0))).astype(f32)
        adpP = np.zeros((NP_, NP_), f32)
        adpP[:N, :N] = adp
        ident = np.eye(128, dtype=f32)
        dpad = np.zeros((NP_,), f32)
        dpad[:N] = dinv
        dmat = dpad.reshape(8, 128).T.copy()
        ones2 = np.zeros((128, 2), f32)
        ones2[:, 0] = 1.0
        ones2[:104, 1] = 1.0
        orow = np.ones((1, 128), f32)
        wcv = np.concatenate([_fold_conv(d, l) for l in range(L)], axis=0)
        wpj = np.concatenate([_fold_proj(d, l) for l in range(L)], axis=0)
        # start conv -> channel-major padded bf16
        x0 = np.einsum("oi,bint->bont", d["start_w"], d["input"],
                       optimize=True).astype(f32)
        xc_all = np.zeros((B, RC, NP_, 168), bf16)
        xc_all[:, :, :N, :] = x0.astype(bf16)
        xc_all = xc_all.reshape(B * RC, NP_ * 168)
        r = _device_forward(xc_all, ident.astype(bf16), adpP.astype(bf16),
                            dmat, ones2, orow,
                            wcv.astype(bf16), wpj.astype(bf16))
        if r is not None:
            # r [8000, 8*147] -> [B, 1000, 8, 147]
            xf = np.ascontiguousarray(
                r.reshape(B, N, RC, 147).transpose(0, 2, 1, 3)
            ).astype(f32)
    if xf is None:
        xf = _host_fallback(d, adp)
    return _pool(xf)

